# revision 18
# baseline (speedup 1.0000x reference)
"""AdaptiveEmbedding T2I sims kernel for 8 TRN2 NeuronCores. v4.1.

Strategy: shard the caption batch (48 -> 6 per core). Each core holds the
full image tensor in [d, i, r] layout, computes BN stats, FiLM params for
its 6 captions, the fovea-softmax weighted pooling, and a [48, 6] slice of
the sims matrix. Host assembles the 8 column slices.

Main loop per (caption, d-block), big tiles [128, 48, 36] bf16:
- ScalarE: e = Exp(s*x + bias), bias = K - |s|*maxabs_d (exact overflow
  guard, no clamp); also the eps-add and u^2 of the batched epilogue.
- Vector: p = e*x (bf16 2x), one combined segmented reduce
  [128, 2*48, 18] -> [128, 2, 48] over the prefolded e|p pair tile.
- GpSimd: r-halving folds of e and p into the shared pair tile.
Prologue built for overlap: DMA dispatch split across both HWDGE queues,
capT columns produced directly by per-(c,blk) PE matmuls (no transpose
chain), caption norms from the diagonal of capT^T capT, BN stats half on
DVE (bn_stats/bn_aggr) and half on ScalarE (activation accum), FiLM
gamma-side matmuls separated from beta-side so the main loop starts as
soon as gamma/scale/bias params exist.
"""

import numpy as np
from contextlib import ExitStack

B, T, D, R = 48, 50, 1024, 36
NCORES = 8
CPC = B // NCORES  # captions per core
SMOOTH = 10.0
KSHIFT = 80.0
BN_EPS = 1e-5
L2_EPS = 1e-8
EPS_S = 1e-37
P = 128
NBLK = D // P          # 8 d-blocks
NIR = B * R            # 1728 rows
RH = R // 2            # 18
NBN = 4                # blocks using DVE bn_stats (rest use ScalarE accum)

_CACHE = {}


def _build_nc():
    import concourse.bass as bass
    import concourse.tile as tile
    from concourse import bacc, mybir
    from concourse.masks import make_identity

    FP = mybir.dt.float32
    BF = mybir.dt.bfloat16
    Alu = mybir.AluOpType
    Act = mybir.ActivationFunctionType

    nc = bacc.Bacc("TRN2", target_bir_lowering=False, debug=False,
                   num_devices=NCORES)

    imgbf = nc.dram_tensor("imgbf", (NIR, D), BF, kind="ExternalInput").ap()
    cap = nc.dram_tensor("cap", (CPC, T, D), BF, kind="ExternalInput").ap()
    maskT_d = nc.dram_tensor("maskT", (T, CPC), BF, kind="ExternalInput").ap()
    wgT_d = nc.dram_tensor("wgT", (D, D), BF, kind="ExternalInput").ap()
    wbT_d = nc.dram_tensor("wbT", (D, D), BF, kind="ExternalInput").ap()
    bg1T_d = nc.dram_tensor("bg1T", (P, NBLK), FP, kind="ExternalInput").ap()
    bbT_d = nc.dram_tensor("bbT", (P, NBLK), FP, kind="ExternalInput").ap()
    out_d = nc.dram_tensor("out", (CPC, B), FP, kind="ExternalOutput").ap()

    with tile.TileContext(nc) as tc, ExitStack() as ctx:
        consts = ctx.enter_context(tc.tile_pool(name="consts", bufs=1))
        ident = consts.tile([P, P], FP, tag="ident")
        make_identity(nc, ident[:])
        ones1 = consts.tile([P, 1], FP, tag="ones1")
        nc.vector.memset(ones1[:], 1.0)
        eps_col = consts.tile([P, 1], FP, tag="eps_col")
        nc.vector.memset(eps_col[:], EPS_S)

        xall_pool = ctx.enter_context(tc.tile_pool(name="xall", bufs=1))
        xall = [xall_pool.tile([P, B, R], BF, tag=f"xall{b}", name=f"xall{b}")
                for b in range(NBLK)]

        smalls = ctx.enter_context(tc.tile_pool(name="smalls", bufs=1))

        # ===== Stage A: input DMA, split across both HWDGE queues =====
        # Sync: img transposes (even blks), W_gamma. Scalar-queue: mask,
        # captions, img transposes (odd blks), W_beta.
        maskT = smalls.tile([T, CPC], BF, tag="maskT")
        nc.scalar.dma_start(out=maskT[:], in_=maskT_d[:, :])
        cap_pool = ctx.enter_context(tc.tile_pool(name="cap", bufs=1))
        cap_tiles = []
        for c in range(CPC):
            ct = cap_pool.tile([T, D], BF, tag=f"cap{c}", name=f"cap{c}")
            nc.scalar.dma_start(out=ct[:], in_=cap[c, :, :])
            cap_tiles.append(ct)
        for blk in range(NBLK):
            nc.sync.dma_start_transpose(
                out=xall[blk][:].rearrange("p i r -> p (i r)"),
                in_=imgbf[:, P * blk:P * (blk + 1)])
        w_pool = ctx.enter_context(tc.tile_pool(name="w", bufs=4))
        wb_pool = ctx.enter_context(tc.tile_pool(name="wbp", bufs=1))
        wg_tiles, wb_tiles = [], []
        for kb in range(NBLK):
            wg = w_pool.tile([P, D], BF, tag="wg", name=f"wg{kb}")
            nc.sync.dma_start(out=wg[:], in_=wgT_d[P * kb:P * (kb + 1), :])
            wg_tiles.append(wg)
        for kb in range(NBLK):
            wb = wb_pool.tile([P, D], BF, tag=f"wb{kb}", name=f"wb{kb}")
            nc.scalar.dma_start(out=wb[:], in_=wbT_d[P * kb:P * (kb + 1), :])
            wb_tiles.append(wb)
        bg1T = smalls.tile([P, NBLK], FP, tag="bg1T")
        nc.sync.dma_start(out=bg1T[:], in_=bg1T_d[:, :])
        bbT = smalls.tile([P, NBLK], FP, tag="bbT")
        nc.sync.dma_start(out=bbT[:], in_=bbT_d[:, :])

        # ===== Stage C': capT columns directly via PE =====
        capT = [smalls.tile([P, CPC], FP, tag=f"capT{b}", name=f"capT{b}")
                for b in range(NBLK)]
        capTb = [smalls.tile([P, CPC], BF, tag=f"capTb{b}", name=f"capTb{b}")
                 for b in range(NBLK)]
        tp_psum = ctx.enter_context(tc.tile_pool(name="tp_ps", bufs=2,
                                                 space="PSUM"))
        with tc.tile_pool(name="capt_ps", bufs=2, space="PSUM") as capt_ps:
            for blk in range(NBLK):
                pc = capt_ps.tile([P, CPC], FP, tag="pc", name="pc")
                for c in range(CPC):
                    nc.tensor.matmul(pc[:, c:c + 1],
                                     cap_tiles[c][:, P * blk:P * (blk + 1)],
                                     maskT[:, c:c + 1],
                                     start=True, stop=True,
                                     skip_group_check=True)
                nc.vector.tensor_copy(out=capT[blk][:], in_=pc[:])
                nc.vector.tensor_copy(out=capTb[blk][:], in_=pc[:])

        # caption norms: diag(capT^T capT) accumulated over blocks
        nsq_sb = smalls.tile([CPC, CPC], FP, tag="nsq_sb")
        with tc.tile_pool(name="nrm_ps", bufs=1, space="PSUM") as nrm_ps:
            pn = nrm_ps.tile([CPC, CPC], FP, tag="pn", name="pn")
            for blk in range(NBLK):
                nc.tensor.matmul(pn[:], capT[blk][:], capT[blk][:],
                                 start=(blk == 0), stop=(blk == NBLK - 1),
                                 skip_group_check=True)
            nc.vector.tensor_copy(out=nsq_sb[:], in_=pn[:])
        nsq_m = smalls.tile([CPC, CPC], FP, tag="nsq_m")
        nc.vector.tensor_tensor(out=nsq_m[:], in0=nsq_sb[:],
                                in1=ident[0:CPC, 0:CPC], op=Alu.mult)
        n2 = smalls.tile([CPC, 1], FP, tag="n2")
        nc.vector.tensor_reduce(out=n2[:], in_=nsq_m[:],
                                axis=mybir.AxisListType.X, op=Alu.add)
        nrm = smalls.tile([CPC, 1], FP, tag="nrm")
        nc.scalar.activation(nrm[:], n2[:], Act.Sqrt)
        nrm_e = smalls.tile([CPC, 1], FP, tag="nrm_e")
        nc.vector.tensor_scalar(out=nrm_e[:], in0=nrm[:], scalar1=L2_EPS,
                                scalar2=None, op0=Alu.add)
        rn = smalls.tile([CPC, 1], FP, tag="rn")
        nc.vector.reciprocal(rn[:], nrm_e[:])

        # ===== Stage B: stats via ScalarE accum, per-half finishing =====
        muT = smalls.tile([P, NBLK], FP, tag="muT")
        varT = smalls.tile([P, NBLK], FP, tag="varT")
        sxT = smalls.tile([P, NBLK], FP, tag="sxT")
        sx2T = smalls.tile([P, NBLK], FP, tag="sx2T")
        maxT = smalls.tile([P, NBLK], FP, tag="maxT")
        rhoT = smalls.tile([P, NBLK], FP, tag="rhoT")
        negmaxT = smalls.tile([P, NBLK], FP, tag="negmaxT")
        stat_pool = ctx.enter_context(tc.tile_pool(name="stat", bufs=2))
        inv_n = 1.0 / float(NIR)
        for half in range(2):
            lo, hi = half * NBN, half * NBN + NBN
            sl = slice(lo, hi)
            for blk in range(lo, hi):
                scr = stat_pool.tile([P, B, R], BF, tag="scr")
                nc.scalar.activation(scr[:], xall[blk][:], Act.Identity,
                                     accum_out=sxT[:, blk:blk + 1])
                scr2 = stat_pool.tile([P, B, R], BF, tag="scr")
                nc.scalar.activation(scr2[:], xall[blk][:], Act.Square,
                                     accum_out=sx2T[:, blk:blk + 1])
                nc.vector.tensor_reduce(
                    out=maxT[:, blk:blk + 1],
                    in_=xall[blk][:].rearrange("p i r -> p (i r)"),
                    axis=mybir.AxisListType.X, op=Alu.max,
                    apply_absolute_value=True)
            nc.vector.tensor_scalar(out=muT[:, sl], in0=sxT[:, sl],
                                    scalar1=inv_n, scalar2=None, op0=Alu.mult)
            m2T = smalls.tile([P, NBN], FP, tag=f"m2T{half}")
            nc.vector.tensor_scalar(out=m2T[:], in0=sx2T[:, sl],
                                    scalar1=inv_n, scalar2=None, op0=Alu.mult)
            musqT = smalls.tile([P, NBN], FP, tag=f"musqT{half}")
            nc.vector.tensor_tensor(out=musqT[:], in0=muT[:, sl],
                                    in1=muT[:, sl], op=Alu.mult)
            nc.vector.tensor_tensor(out=varT[:, sl], in0=m2T[:], in1=musqT[:],
                                    op=Alu.subtract)
            varTe = smalls.tile([P, NBN], FP, tag=f"varTe{half}")
            nc.vector.tensor_scalar(out=varTe[:], in0=varT[:, sl],
                                    scalar1=BN_EPS, scalar2=None, op0=Alu.add)
            stdT = smalls.tile([P, NBN], FP, tag=f"stdT{half}")
            nc.scalar.activation(stdT[:], varTe[:], Act.Sqrt)
            nc.vector.reciprocal_approx_fast(rhoT[:, sl], stdT[:])
            nc.vector.tensor_scalar(out=negmaxT[:, sl], in0=maxT[:, sl],
                                    scalar1=-1.0, scalar2=None, op0=Alu.mult)

        # ===== Stage D: FiLM matmuls, gamma side first =====
        gamT = smalls.tile([P, NBLK, CPC], FP, tag="gamT")
        betT = smalls.tile([P, NBLK, CPC], FP, tag="betT")
        aT = smalls.tile([P, NBLK, CPC], FP, tag="aT")
        b2T = smalls.tile([P, NBLK, CPC], FP, tag="b2T")
        scaleT = smalls.tile([P, NBLK, CPC], FP, tag="scaleT")
        biasT = smalls.tile([P, NBLK, CPC], FP, tag="biasT")

        def bcast(colT):
            return colT[:].unsqueeze(2).broadcast_to((P, NBLK, CPC))

        gcd_pool = ctx.enter_context(tc.tile_pool(name="gcd", bufs=2))
        with tc.tile_pool(name="gb_ps", bufs=1, space="PSUM") as gb_ps_pool:
            for wkey, wtiles, dest in (("g", wg_tiles, gamT),
                                       ("b", wb_tiles, betT)):
                psh = [gb_ps_pool.tile([CPC, 512], FP, tag=f"ps_{wkey}{h}",
                                       name=f"ps_{wkey}{h}")
                       for h in range(2)]
                for kb in range(NBLK):
                    for half in range(2):
                        nc.tensor.matmul(psh[half][:], capTb[kb][:],
                                         wtiles[kb][:, 512 * half:
                                                    512 * (half + 1)],
                                         start=(kb == 0),
                                         stop=(kb == NBLK - 1),
                                         skip_group_check=True)
                for half in range(2):
                    gsb = gcd_pool.tile([CPC, 512], FP, tag="gsb",
                                        name=f"gsb{wkey}{half}")
                    nc.vector.tensor_copy(out=gsb[:], in_=psh[half][:])
                    for j in range(4):
                        db = half * 4 + j
                        pst = tp_psum.tile([P, P], FP, tag="tp")
                        nc.tensor.transpose(pst[:, 0:CPC],
                                            gsb[:, P * j:P * (j + 1)],
                                            ident[:CPC, :CPC])
                        nc.vector.tensor_copy(out=dest[:, db, :],
                                              in_=pst[:, 0:CPC])
                if wkey == "g":
                    # gamma-side batched small-ops -> scaleT/biasT/aT,
                    # per block-half so the main loop can start early
                    for bh in range(2):
                        bsl = slice(bh * NBN, bh * NBN + NBN)

                        def bc4(colT, bsl=bsl):
                            return colT[:, bsl].unsqueeze(2).broadcast_to(
                                (P, NBN, CPC))

                        gp1 = smalls.tile([P, NBN, CPC], FP, tag=f"gp1{bh}")
                        nc.vector.tensor_tensor(out=gp1[:], in0=gamT[:, bsl, :],
                                                in1=bc4(bg1T), op=Alu.add)
                        nc.vector.tensor_tensor(out=aT[:, bsl, :], in0=gp1[:],
                                                in1=bc4(rhoT), op=Alu.mult)
                        nc.vector.tensor_scalar(
                            out=scaleT[:, bsl, :], in0=aT[:, bsl, :],
                            scalar1=SMOOTH, scalar2=None, op0=Alu.mult)
                        negsc = smalls.tile([P, NBN, CPC], FP,
                                            tag=f"negsc{bh}")
                        nc.vector.tensor_scalar(
                            out=negsc[:], in0=scaleT[:, bsl, :],
                            scalar1=-1.0, scalar2=None, op0=Alu.mult)
                        absS = smalls.tile([P, NBN, CPC], FP, tag=f"absS{bh}")
                        nc.vector.tensor_tensor(out=absS[:],
                                                in0=scaleT[:, bsl, :],
                                                in1=negsc[:], op=Alu.max)
                        bias0 = smalls.tile([P, NBN, CPC], FP,
                                            tag=f"bias0{bh}")
                        nc.vector.tensor_tensor(out=bias0[:], in0=absS[:],
                                                in1=bc4(negmaxT), op=Alu.mult)
                        nc.vector.tensor_scalar(
                            out=biasT[:, bsl, :], in0=bias0[:],
                            scalar1=KSHIFT, scalar2=None, op0=Alu.add)

        # beta-side batched small-ops -> b2T
        bet1 = smalls.tile([P, NBLK, CPC], FP, tag="bet1")
        nc.vector.tensor_tensor(out=bet1[:], in0=betT[:], in1=bcast(bbT),
                                op=Alu.add)
        amu = smalls.tile([P, NBLK, CPC], FP, tag="amu")
        nc.vector.tensor_tensor(out=amu[:], in0=aT[:], in1=bcast(muT),
                                op=Alu.mult)
        nc.vector.tensor_tensor(out=b2T[:], in0=bet1[:], in1=amu[:],
                                op=Alu.subtract)

        # ===== Stage E: main loop =====
        ep_pool = ctx.enter_context(tc.tile_pool(name="ep", bufs=5))
        f_pool = ctx.enter_context(tc.tile_pool(name="f", bufs=5))
        sw_pool = ctx.enter_context(tc.tile_pool(name="sw", bufs=2))
        sc_pool = ctx.enter_context(tc.tile_pool(name="sc", bufs=2))
        dots_sb = smalls.tile([CPC, B], FP, tag="dots_sb")
        usq_sb = smalls.tile([CPC, B], FP, tag="usq_sb")
        with tc.tile_pool(name="dot_ps", bufs=2, space="PSUM") as dot_ps_pool:
            for c in range(CPC):
                # sums tile: [P, NBLK, 2, B]; [:, blk, 0/1, :] = sum e / sum p
                sw_big = sw_pool.tile([P, NBLK, 2, B], FP, tag="swb")
                for blk in range(NBLK):
                    ep = ep_pool.tile([P, 2, B, R], BF, tag="ep")
                    nc.scalar.activation(ep[:, 0, :, :], xall[blk][:], Act.Exp,
                                         scale=scaleT[:, blk, c:c + 1],
                                         bias=biasT[:, blk, c:c + 1])
                    nc.vector.tensor_tensor(out=ep[:, 1, :, :],
                                            in0=ep[:, 0, :, :],
                                            in1=xall[blk][:], op=Alu.mult)
                    f = f_pool.tile([P, 2, B, RH], BF, tag="f")
                    nc.gpsimd.tensor_tensor(out=f[:],
                                            in0=ep[:, :, :, 0:RH],
                                            in1=ep[:, :, :, RH:R], op=Alu.add)
                    nc.vector.tensor_reduce(
                        out=sw_big[:, blk, :, :].rearrange("p s b -> p (s b)"),
                        in_=f[:].rearrange("p s b r -> p (s b) r"),
                        axis=mybir.AxisListType.X, op=Alu.add)

                # batched per-c epilogue on [P, NBLK, B]
                ssum_v = sw_big[:, :, 0, :]
                wsum_v = sw_big[:, :, 1, :]
                sse = sc_pool.tile([P, NBLK, B], FP, tag="sse")
                nc.scalar.activation(sse[:], ssum_v, Act.Identity,
                                     bias=eps_col[:, 0:1])
                rs = sc_pool.tile([P, NBLK, B], FP, tag="rs")
                nc.vector.reciprocal_approx_fast(
                    rs[:].rearrange("p a b -> p (a b)"),
                    sse[:].rearrange("p a b -> p (a b)"))
                wr = sc_pool.tile([P, NBLK, B], FP, tag="wr")
                nc.vector.tensor_tensor(out=wr[:], in0=wsum_v, in1=rs[:],
                                        op=Alu.mult)
                wa = sc_pool.tile([P, NBLK, B], FP, tag="wa")
                nc.vector.tensor_tensor(
                    out=wa[:], in0=wr[:],
                    in1=aT[:, :, c].unsqueeze(2).broadcast_to((P, NBLK, B)),
                    op=Alu.mult)
                u = sc_pool.tile([P, NBLK, B], FP, tag="u")
                nc.vector.tensor_tensor(
                    out=u[:], in0=wa[:],
                    in1=b2T[:, :, c].unsqueeze(2).broadcast_to((P, NBLK, B)),
                    op=Alu.add)
                uu = sc_pool.tile([P, NBLK, B], FP, tag="uu")
                nc.scalar.activation(uu[:], u[:], Act.Square)

                ps_dot = dot_ps_pool.tile([1, B], FP, tag="dot")
                ps_usq = dot_ps_pool.tile([1, B], FP, tag="usq")
                for blk in range(NBLK):
                    nc.tensor.matmul(ps_dot[:], capT[blk][:, c:c + 1],
                                     u[:, blk, :],
                                     start=(blk == 0), stop=(blk == NBLK - 1),
                                     skip_group_check=True)
                    nc.tensor.matmul(ps_usq[:], ones1[:], uu[:, blk, :],
                                     start=(blk == 0), stop=(blk == NBLK - 1),
                                     skip_group_check=True)
                drow = sc_pool.tile([1, B], FP, tag="drow")
                nc.scalar.copy(drow[:], ps_dot[:])
                urow = sc_pool.tile([1, B], FP, tag="urow")
                nc.scalar.copy(urow[:], ps_usq[:])
                nc.sync.dma_start(out=dots_sb[c:c + 1, :], in_=drow[:])
                nc.sync.dma_start(out=usq_sb[c:c + 1, :], in_=urow[:])

        # ===== Stage F: epilogue =====
        out_sb = smalls.tile([CPC, B], FP, tag="out_sb")
        sq = smalls.tile([CPC, B], FP, tag="sqf")
        nc.scalar.activation(sq[:], usq_sb[:], Act.Sqrt)
        ru = smalls.tile([CPC, B], FP, tag="ruf")
        nc.vector.reciprocal_approx_fast(ru[:], sq[:])
        t1 = smalls.tile([CPC, B], FP, tag="t1f")
        nc.vector.tensor_tensor(out=t1[:], in0=dots_sb[:], in1=ru[:],
                                op=Alu.mult)
        nc.vector.tensor_scalar(out=out_sb[:], in0=t1[:],
                                scalar1=rn[:, 0:1], scalar2=None, op0=Alu.mult)
        nc.sync.dma_start(out=out_d[:, :], in_=out_sb[:])

    nc.compile()
    return nc


def _get_nc():
    if "nc" not in _CACHE:
        _CACHE["nc"] = _build_nc()
    return _CACHE["nc"]


def kernel(img_embed, cap_embed, lens, W_gamma, b_gamma, W_beta, b_beta,
           _want_trace=False):
    from concourse.bass_utils import run_bass_kernel_spmd

    nc = _get_nc()

    img_embed = np.asarray(img_embed, np.float32)
    cap_embed = np.asarray(cap_embed, np.float32)
    lens_np = np.asarray(lens)
    W_gamma = np.asarray(W_gamma, np.float32)
    W_beta = np.asarray(W_beta, np.float32)
    b_gamma = np.asarray(b_gamma, np.float32)
    b_beta = np.asarray(b_beta, np.float32)

    import ml_dtypes
    img_bf = np.ascontiguousarray(
        img_embed.reshape(NIR, D).astype(ml_dtypes.bfloat16))
    wgT = np.ascontiguousarray(W_gamma.T.astype(ml_dtypes.bfloat16))
    wbT = np.ascontiguousarray(W_beta.T.astype(ml_dtypes.bfloat16))
    bg1T = np.ascontiguousarray((1.0 + b_gamma).reshape(NBLK, P).T)
    bbT = np.ascontiguousarray(b_beta.reshape(NBLK, P).T)

    lens_f = lens_np.astype(np.float64)
    mask = (np.arange(T)[None, :] < lens_np[:, None]).astype(np.float64)
    mask = (mask / lens_f[:, None]).astype(np.float32)  # (B, T)

    in_maps = []
    for k in range(NCORES):
        sl = slice(k * CPC, (k + 1) * CPC)
        in_maps.append({
            "imgbf": img_bf,
            "cap": np.ascontiguousarray(
                cap_embed[sl].astype(ml_dtypes.bfloat16)),
            "maskT": np.ascontiguousarray(
                mask[sl].T.astype(ml_dtypes.bfloat16)),
            "wgT": wgT,
            "wbT": wbT,
            "bg1T": bg1T,
            "bbT": bbT,
        })

    kw = {}
    if _want_trace:
        import os as _os2, shutil as _sh
        _sh.rmtree("/tmp/ktrace", ignore_errors=True)
        _os2.makedirs("/tmp/ktrace", exist_ok=True)
        kw = {"tmpdir": "/tmp/ktrace"}
    res = run_bass_kernel_spmd(nc, in_maps, core_ids=list(range(NCORES)),
                               trace=_want_trace, **kw)
    outs = [np.asarray(r["out"]) for r in res.results]
    sims = np.concatenate([o.T for o in outs], axis=1).astype(np.float32)
    if _want_trace:
        return sims, res
    return sims


# revision 20
# speedup vs baseline: 1.0989x; 1.0989x over previous
"""AdaptiveEmbedding T2I sims kernel for 8 TRN2 NeuronCores. v3.

Strategy: shard the caption batch (48 -> 6 per core). Each core holds the
full image tensor in [d, i, r] layout, computes BN stats, FiLM params for
its 6 captions, the fovea-softmax weighted pooling, and a [48, 6] slice of
the sims matrix. Host assembles the 8 column slices.

Engine assignment per (caption, d-block) iteration, all on [128, 48, 36]
bf16 tiles:
- ScalarE: e = Exp(s*x + bias). No clamp needed: bias = K - |s|*maxabs_d
  guarantees the exponent <= K=80 < 88.7 (fp32 exp overflow). Rows that
  fully underflow (|s|*gap > ~170, ~1e-5 of cases) are rescued by an eps
  on sum(e) and degrade to u=b2 instead of NaN.
- Vector: p = e*x multiply (bf16 2x mode), then two half-size segmented
  reduces [128,48,18]->[128,48] over the GpSimd-prefolded tiles.
- GpSimd: r-halving folds e[...,0:18]+e[...,18:36] (and same for p) --
  the 2-input elementwise floor, ~1.8us each, freeing Vector cycles.
- Per-caption (not per-block) batched epilogue: eps-add, fast reciprocal,
  u = a*(w/s)+b2 and u^2 on [128, 8*48] tiles with 0-stride broadcast APs
  for the per-(c,blk) FiLM scalars.
- BN stats via ScalarE activation accum_out (Identity -> sum x,
  Square -> sum x^2); only the per-channel maxabs reduce uses Vector.
"""

import numpy as np
from contextlib import ExitStack

B, T, D, R = 48, 50, 1024, 36
NCORES = 8
CPC = B // NCORES  # captions per core
SMOOTH = 10.0
KSHIFT = 80.0
BN_EPS = 1e-5
L2_EPS = 1e-8
EPS_S = 1e-37
P = 128
NBLK = D // P          # 8 d-blocks
NIR = B * R            # 1728 rows
RH = R // 2            # 18

_CACHE = {}


def _build_nc():
    import concourse.bass as bass
    import concourse.tile as tile
    from concourse import bacc, mybir
    from concourse.masks import make_identity

    FP = mybir.dt.float32
    BF = mybir.dt.bfloat16
    Alu = mybir.AluOpType
    Act = mybir.ActivationFunctionType

    nc = bacc.Bacc("TRN2", target_bir_lowering=False, debug=False,
                   num_devices=NCORES)

    imgbf = nc.dram_tensor("imgbf", (NIR, D), BF, kind="ExternalInput").ap()
    cap = nc.dram_tensor("cap", (CPC, T, D), BF, kind="ExternalInput").ap()
    maskT_d = nc.dram_tensor("maskT", (T, CPC), BF, kind="ExternalInput").ap()
    wgT_d = nc.dram_tensor("wgT", (D, D), BF, kind="ExternalInput").ap()
    wbT_d = nc.dram_tensor("wbT", (D, D), BF, kind="ExternalInput").ap()
    bg1T_d = nc.dram_tensor("bg1T", (P, NBLK), FP, kind="ExternalInput").ap()
    bbT_d = nc.dram_tensor("bbT", (P, NBLK), FP, kind="ExternalInput").ap()
    out_d = nc.dram_tensor("out", (CPC, B), FP, kind="ExternalOutput").ap()

    with tile.TileContext(nc) as tc, ExitStack() as ctx:
        consts = ctx.enter_context(tc.tile_pool(name="consts", bufs=1))
        ident = consts.tile([P, P], FP, tag="ident")
        make_identity(nc, ident[:])
        ones1 = consts.tile([P, 1], FP, tag="ones1")
        nc.vector.memset(ones1[:], 1.0)
        eps_col = consts.tile([P, 1], FP, tag="eps_col")
        nc.vector.memset(eps_col[:], EPS_S)

        xall_pool = ctx.enter_context(tc.tile_pool(name="xall", bufs=1))
        xall = [xall_pool.tile([P, B, R], BF, tag=f"xall{b}", name=f"xall{b}")
                for b in range(NBLK)]

        smalls = ctx.enter_context(tc.tile_pool(name="smalls", bufs=1))
        tp_psum = ctx.enter_context(tc.tile_pool(name="tp_ps", bufs=2,
                                                 space="PSUM"))

        # ========== Stage A: DMA-transpose img (bf16, [d, i, r]) ==========
        for blk in range(NBLK):
            nc.sync.dma_start_transpose(
                out=xall[blk][:].rearrange("p i r -> p (i r)"),
                in_=imgbf[:, P * blk:P * (blk + 1)])

        # ========== Stage B: stats. ScalarE accum sums, DVE maxabs ========
        sxT = smalls.tile([P, NBLK], FP, tag="sxT")
        sx2T = smalls.tile([P, NBLK], FP, tag="sx2T")
        maxT = smalls.tile([P, NBLK], FP, tag="maxT")
        stat_pool = ctx.enter_context(tc.tile_pool(name="stat", bufs=2))
        for blk in range(NBLK):
            scr = stat_pool.tile([P, B, R], BF, tag="scr")
            nc.scalar.activation(scr[:], xall[blk][:], Act.Identity,
                                 accum_out=sxT[:, blk:blk + 1])
            scr2 = stat_pool.tile([P, B, R], BF, tag="scr")
            nc.scalar.activation(scr2[:], xall[blk][:], Act.Square,
                                 accum_out=sx2T[:, blk:blk + 1])
            nc.vector.tensor_reduce(
                out=maxT[:, blk:blk + 1],
                in_=xall[blk][:].rearrange("p i r -> p (i r)"),
                axis=mybir.AxisListType.X, op=Alu.max,
                apply_absolute_value=True)

        inv_n = 1.0 / float(NIR)
        muT = smalls.tile([P, NBLK], FP, tag="muT")
        nc.vector.tensor_scalar(out=muT[:], in0=sxT[:], scalar1=inv_n,
                                scalar2=None, op0=Alu.mult)
        m2T = smalls.tile([P, NBLK], FP, tag="m2T")
        nc.vector.tensor_scalar(out=m2T[:], in0=sx2T[:], scalar1=inv_n,
                                scalar2=None, op0=Alu.mult)
        musqT = smalls.tile([P, NBLK], FP, tag="musqT")
        nc.vector.tensor_tensor(out=musqT[:], in0=muT[:], in1=muT[:],
                                op=Alu.mult)
        varT = smalls.tile([P, NBLK], FP, tag="varT")
        nc.vector.tensor_tensor(out=varT[:], in0=m2T[:], in1=musqT[:],
                                op=Alu.subtract)
        varTe = smalls.tile([P, NBLK], FP, tag="varTe")
        nc.vector.tensor_scalar(out=varTe[:], in0=varT[:], scalar1=BN_EPS,
                                scalar2=None, op0=Alu.add)
        stdT = smalls.tile([P, NBLK], FP, tag="stdT")
        nc.scalar.activation(stdT[:], varTe[:], Act.Sqrt)
        rhoT = smalls.tile([P, NBLK], FP, tag="rhoT")
        nc.vector.reciprocal_approx_fast(rhoT[:], stdT[:])
        negmaxT = smalls.tile([P, NBLK], FP, tag="negmaxT")
        nc.vector.tensor_scalar(out=negmaxT[:], in0=maxT[:], scalar1=-1.0,
                                scalar2=None, op0=Alu.mult)

        # ========== Stage C: caption pooling + capT + norms ==========
        maskT = smalls.tile([T, CPC], BF, tag="maskT")
        nc.sync.dma_start(out=maskT[:], in_=maskT_d[:, :])
        cap_pool = ctx.enter_context(tc.tile_pool(name="cap", bufs=2))
        cap_sb = smalls.tile([CPC, D], FP, tag="cap_sb")
        with tc.tile_pool(name="cap_ps", bufs=2, space="PSUM") as cap_ps_pool:
            for c in range(CPC):
                ct = cap_pool.tile([T, D], BF, tag="cap")
                nc.sync.dma_start(out=ct[:], in_=cap[c, :, :])
                pp = cap_ps_pool.tile([1, D], FP, tag="pp", name="pp")
                for j in range(2):
                    nc.tensor.matmul(pp[:, 512 * j:512 * (j + 1)],
                                     maskT[:, c:c + 1],
                                     ct[:, 512 * j:512 * (j + 1)],
                                     start=True, stop=True,
                                     skip_group_check=True)
                prow = cap_pool.tile([1, D], FP, tag="prow", name="prow",
                                     bufs=2)
                nc.scalar.copy(prow[:], pp[:])
                nc.sync.dma_start(out=cap_sb[c:c + 1, :], in_=prow[:])

        capT = [smalls.tile([P, CPC], FP, tag=f"capT{b}", name=f"capT{b}")
                for b in range(NBLK)]
        capTb = [smalls.tile([P, CPC], BF, tag=f"capTb{b}", name=f"capTb{b}")
                 for b in range(NBLK)]
        for blk in range(NBLK):
            pst = tp_psum.tile([P, P], FP, tag="tp")
            nc.tensor.transpose(pst[:, 0:CPC], cap_sb[:, P * blk:P * (blk + 1)],
                                ident[:CPC, :CPC])
            nc.vector.tensor_copy(out=capT[blk][:], in_=pst[:, 0:CPC])
            nc.vector.tensor_copy(out=capTb[blk][:], in_=pst[:, 0:CPC])

        scr_c = smalls.tile([CPC, D], FP, tag="scr_c")
        n2 = smalls.tile([CPC, 1], FP, tag="n2")
        nc.vector.tensor_tensor(out=scr_c[:], in0=cap_sb[:], in1=cap_sb[:],
                                op=Alu.mult)
        nc.vector.tensor_reduce(out=n2[:], in_=scr_c[:],
                                axis=mybir.AxisListType.X, op=Alu.add)
        nrm = smalls.tile([CPC, 1], FP, tag="nrm")
        nc.scalar.activation(nrm[:], n2[:], Act.Sqrt)
        nrm_e = smalls.tile([CPC, 1], FP, tag="nrm_e")
        nc.vector.tensor_scalar(out=nrm_e[:], in0=nrm[:], scalar1=L2_EPS,
                                scalar2=None, op0=Alu.add)
        rn = smalls.tile([CPC, 1], FP, tag="rn")
        nc.vector.reciprocal(rn[:], nrm_e[:])

        # ========== Stage D: FiLM params, capT-stationary ==========
        bg1T = smalls.tile([P, NBLK], FP, tag="bg1T")
        nc.sync.dma_start(out=bg1T[:], in_=bg1T_d[:, :])
        bbT = smalls.tile([P, NBLK], FP, tag="bbT")
        nc.sync.dma_start(out=bbT[:], in_=bbT_d[:, :])

        # packed [P, NBLK, CPC] param tiles for broadcast slicing
        aT = smalls.tile([P, NBLK, CPC], FP, tag="aT")
        b2T = smalls.tile([P, NBLK, CPC], FP, tag="b2T")
        scaleT = smalls.tile([P, NBLK, CPC], FP, tag="scaleT")
        biasT = smalls.tile([P, NBLK, CPC], FP, tag="biasT")

        w_pool = ctx.enter_context(tc.tile_pool(name="w", bufs=3))
        gcd_pool = ctx.enter_context(tc.tile_pool(name="gcd", bufs=2))
        with tc.tile_pool(name="gb_ps", bufs=4, space="PSUM") as gb_ps_pool:
            for which, wd in (("g", wgT_d), ("b", wbT_d)):
                for half in range(2):
                    ps = gb_ps_pool.tile([CPC, 512], FP, tag="gcd",
                                         name="gcd_ps")
                    for kb in range(NBLK):
                        w = w_pool.tile([P, D // 2], BF, tag="w", name="w")
                        nc.sync.dma_start(
                            out=w[:], in_=wd[P * kb:P * (kb + 1),
                                             512 * half:512 * (half + 1)])
                        nc.tensor.matmul(ps[:], capTb[kb][:], w[:],
                                         start=(kb == 0),
                                         stop=(kb == NBLK - 1),
                                         skip_group_check=True)
                    gsb = gcd_pool.tile([CPC, 512], FP, tag="gsb", name="gsb")
                    nc.scalar.copy(gsb[:], ps[:])
                    for j in range(4):
                        db = half * 4 + j
                        pst = tp_psum.tile([P, P], FP, tag="tp")
                        nc.tensor.transpose(pst[:, 0:CPC],
                                            gsb[:, P * j:P * (j + 1)],
                                            ident[:CPC, :CPC])
                        if which == "g":
                            gp1 = smalls.tile([P, CPC], FP, tag=f"gp1_{db}",
                                              name=f"gp1_{db}")
                            nc.vector.tensor_scalar(out=gp1[:],
                                                    in0=pst[:, 0:CPC],
                                                    scalar1=bg1T[:, db:db + 1],
                                                    scalar2=None, op0=Alu.add)
                            nc.vector.tensor_scalar(out=aT[:, db, :], in0=gp1[:],
                                                    scalar1=rhoT[:, db:db + 1],
                                                    scalar2=None, op0=Alu.mult)
                            nc.vector.tensor_scalar(out=scaleT[:, db, :],
                                                    in0=aT[:, db, :],
                                                    scalar1=SMOOTH,
                                                    scalar2=None, op0=Alu.mult)
                            negsc = smalls.tile([P, CPC], FP, tag="negsc")
                            nc.vector.tensor_scalar(out=negsc[:],
                                                    in0=scaleT[:, db, :],
                                                    scalar1=-1.0, scalar2=None,
                                                    op0=Alu.mult)
                            absS = smalls.tile([P, CPC], FP, tag="absS")
                            nc.vector.tensor_tensor(out=absS[:],
                                                    in0=scaleT[:, db, :],
                                                    in1=negsc[:], op=Alu.max)
                            nc.vector.tensor_scalar(
                                out=biasT[:, db, :], in0=absS[:],
                                scalar1=negmaxT[:, db:db + 1],
                                scalar2=KSHIFT, op0=Alu.mult, op1=Alu.add)
                        else:
                            betat = smalls.tile([P, CPC], FP, tag=f"bet_{db}",
                                                name=f"bet_{db}")
                            nc.vector.tensor_scalar(out=betat[:],
                                                    in0=pst[:, 0:CPC],
                                                    scalar1=bbT[:, db:db + 1],
                                                    scalar2=None, op0=Alu.add)
                            amu = smalls.tile([P, CPC], FP, tag="amu")
                            nc.vector.tensor_scalar(out=amu[:], in0=aT[:, db, :],
                                                    scalar1=muT[:, db:db + 1],
                                                    scalar2=None, op0=Alu.mult)
                            nc.vector.tensor_tensor(out=b2T[:, db, :],
                                                    in0=betat[:], in1=amu[:],
                                                    op=Alu.subtract)

        # ========== Stage E: main loop ==========
        ep_pool = ctx.enter_context(tc.tile_pool(name="ep", bufs=4))
        f_pool = ctx.enter_context(tc.tile_pool(name="f", bufs=4))
        sw_pool = ctx.enter_context(tc.tile_pool(name="sw", bufs=2))
        sc_pool = ctx.enter_context(tc.tile_pool(name="sc", bufs=3))
        dots_sb = smalls.tile([CPC, B], FP, tag="dots_sb")
        usq_sb = smalls.tile([CPC, B], FP, tag="usq_sb")
        with tc.tile_pool(name="dot_ps", bufs=2, space="PSUM") as dot_ps_pool:
            for c in range(CPC):
                sw_big = sw_pool.tile([P, NBLK, 2, B], FP, tag="swb")
                for blk in range(NBLK):
                    ep = ep_pool.tile([P, 2, B, R], BF, tag="ep")
                    nc.scalar.activation(ep[:, 0, :, :], xall[blk][:], Act.Exp,
                                         scale=scaleT[:, blk, c:c + 1],
                                         bias=biasT[:, blk, c:c + 1])
                    nc.vector.tensor_tensor(out=ep[:, 1, :, :],
                                            in0=ep[:, 0, :, :],
                                            in1=xall[blk][:], op=Alu.mult)
                    f = f_pool.tile([P, 2, B, RH], BF, tag="f")
                    nc.gpsimd.tensor_tensor(out=f[:],
                                            in0=ep[:, :, :, 0:RH],
                                            in1=ep[:, :, :, RH:R], op=Alu.add)
                    nc.vector.tensor_reduce(
                        out=sw_big[:, blk, :, :].rearrange("p s b -> p (s b)"),
                        in_=f[:].rearrange("p s b r -> p (s b) r"),
                        axis=mybir.AxisListType.X, op=Alu.add)

                # batched per-c epilogue on [P, NBLK, B]
                ssum_v = sw_big[:, :, 0, :]
                wsum_v = sw_big[:, :, 1, :]
                sse = sc_pool.tile([P, NBLK, B], FP, tag="sse")
                nc.scalar.activation(sse[:], ssum_v, Act.Identity,
                                     bias=eps_col[:, 0:1])
                rs = sc_pool.tile([P, NBLK, B], FP, tag="rs")
                nc.vector.reciprocal_approx_fast(
                    rs[:].rearrange("p a b -> p (a b)"),
                    sse[:].rearrange("p a b -> p (a b)"))
                wr = sc_pool.tile([P, NBLK, B], FP, tag="wr")
                nc.vector.tensor_tensor(out=wr[:], in0=wsum_v, in1=rs[:],
                                        op=Alu.mult)
                wa = sc_pool.tile([P, NBLK, B], FP, tag="wa")
                nc.vector.tensor_tensor(
                    out=wa[:], in0=wr[:],
                    in1=aT[:, :, c].unsqueeze(2).broadcast_to((P, NBLK, B)),
                    op=Alu.mult)
                u = sc_pool.tile([P, NBLK, B], FP, tag="u")
                nc.vector.tensor_tensor(
                    out=u[:], in0=wa[:],
                    in1=b2T[:, :, c].unsqueeze(2).broadcast_to((P, NBLK, B)),
                    op=Alu.add)
                uu = sc_pool.tile([P, NBLK, B], FP, tag="uu")
                nc.scalar.activation(uu[:], u[:], Act.Square)

                ps_dot = dot_ps_pool.tile([1, B], FP, tag="dot")
                ps_usq = dot_ps_pool.tile([1, B], FP, tag="usq")
                for blk in range(NBLK):
                    nc.tensor.matmul(ps_dot[:], capT[blk][:, c:c + 1],
                                     u[:, blk, :],
                                     start=(blk == 0), stop=(blk == NBLK - 1),
                                     skip_group_check=True)
                    nc.tensor.matmul(ps_usq[:], ones1[:], uu[:, blk, :],
                                     start=(blk == 0), stop=(blk == NBLK - 1),
                                     skip_group_check=True)
                drow = sc_pool.tile([1, B], FP, tag="drow")
                nc.scalar.copy(drow[:], ps_dot[:])
                urow = sc_pool.tile([1, B], FP, tag="urow")
                nc.scalar.copy(urow[:], ps_usq[:])
                nc.sync.dma_start(out=dots_sb[c:c + 1, :], in_=drow[:])
                nc.sync.dma_start(out=usq_sb[c:c + 1, :], in_=urow[:])

        # ========== Stage F: epilogue ==========
        out_sb = smalls.tile([CPC, B], FP, tag="out_sb")
        sq = smalls.tile([CPC, B], FP, tag="sqf")
        nc.scalar.activation(sq[:], usq_sb[:], Act.Sqrt)
        ru = smalls.tile([CPC, B], FP, tag="ruf")
        nc.vector.reciprocal_approx_fast(ru[:], sq[:])
        t1 = smalls.tile([CPC, B], FP, tag="t1f")
        nc.vector.tensor_tensor(out=t1[:], in0=dots_sb[:], in1=ru[:],
                                op=Alu.mult)
        nc.vector.tensor_scalar(out=out_sb[:], in0=t1[:],
                                scalar1=rn[:, 0:1], scalar2=None, op0=Alu.mult)
        nc.sync.dma_start(out=out_d[:, :], in_=out_sb[:])

    nc.compile()
    return nc


def _get_nc():
    if "nc" not in _CACHE:
        _CACHE["nc"] = _build_nc()
    return _CACHE["nc"]


def kernel(img_embed, cap_embed, lens, W_gamma, b_gamma, W_beta, b_beta,
           _want_trace=False):
    from concourse.bass_utils import run_bass_kernel_spmd

    nc = _get_nc()

    img_embed = np.asarray(img_embed, np.float32)
    cap_embed = np.asarray(cap_embed, np.float32)
    lens_np = np.asarray(lens)
    W_gamma = np.asarray(W_gamma, np.float32)
    W_beta = np.asarray(W_beta, np.float32)
    b_gamma = np.asarray(b_gamma, np.float32)
    b_beta = np.asarray(b_beta, np.float32)

    import ml_dtypes
    img_bf = np.ascontiguousarray(
        img_embed.reshape(NIR, D).astype(ml_dtypes.bfloat16))
    wgT = np.ascontiguousarray(W_gamma.T.astype(ml_dtypes.bfloat16))
    wbT = np.ascontiguousarray(W_beta.T.astype(ml_dtypes.bfloat16))
    bg1T = np.ascontiguousarray((1.0 + b_gamma).reshape(NBLK, P).T)
    bbT = np.ascontiguousarray(b_beta.reshape(NBLK, P).T)

    lens_f = lens_np.astype(np.float64)
    mask = (np.arange(T)[None, :] < lens_np[:, None]).astype(np.float64)
    mask = (mask / lens_f[:, None]).astype(np.float32)  # (B, T)

    in_maps = []
    for k in range(NCORES):
        sl = slice(k * CPC, (k + 1) * CPC)
        in_maps.append({
            "imgbf": img_bf,
            "cap": np.ascontiguousarray(
                cap_embed[sl].astype(ml_dtypes.bfloat16)),
            "maskT": np.ascontiguousarray(
                mask[sl].T.astype(ml_dtypes.bfloat16)),
            "wgT": wgT,
            "wbT": wbT,
            "bg1T": bg1T,
            "bbT": bbT,
        })

    kw = {}
    if _want_trace:
        import os as _os2, shutil as _sh
        _sh.rmtree("/tmp/ktrace", ignore_errors=True)
        _os2.makedirs("/tmp/ktrace", exist_ok=True)
        kw = {"tmpdir": "/tmp/ktrace"}
    res = run_bass_kernel_spmd(nc, in_maps, core_ids=list(range(NCORES)),
                               trace=_want_trace, **kw)
    outs = [np.asarray(r["out"]) for r in res.results]
    sims = np.concatenate([o.T for o in outs], axis=1).astype(np.float32)
    if _want_trace:
        return sims, res
    return sims
